# revision 22
# baseline (speedup 1.0000x reference)
"""Distributed Trainium2 kernel for nn_Attention_72722386256499.

Full inputs in, full output out.  Internally shards the 32 (B,H)
attention problems over 8 NeuronCores: core m handles batch m//2,
heads [4*(m%2), 4*(m%2)+4).  The small 1x1-conv weights are sliced and
replicated host-side; the output projection is computed as per-core
partial products summed on the host (data-parallel reduce in unshard).
"""

import sys

sys.path.insert(0, "/opt/trn_rl_repo")

import ml_dtypes
import numpy as np

import bass_rust
import concourse.bass as bass
import concourse.mybir as mybir
import concourse.tile as tile
from concourse import masks
from concourse.bass_utils import run_bass_kernel_spmd

B, C, L = 4, 512, 2048
H, D = 8, 64
HPC = 4  # heads per core
NCORES = 8
FP = mybir.dt.float32

# Matmul compute dtype: float32 (4 cyc/row) or float32r (1 cyc/row @ N>=256).
MM_DT = mybir.dt.bfloat16

TRACE_MODE = False
LAST_RESULT = None
_NC_CACHE = {}


def _split_waits(nc, max_waits=1):
    """walrus here rejects >1 sync wait per instruction; hoist extras onto
    single-wait NoOps just before the instruction on the same engine."""
    counter = 0
    for f in nc.m.functions:
        for bb in f.blocks:
            il = bb.instructions
            new_list = []
            changed = False
            for inst in il:
                si = inst.sync_info
                if si is None:
                    new_list.append(inst)
                    continue
                waits = list(si.on_wait)
                if len(waits) > max_waits:
                    keep = waits[-max_waits:]
                    for w in waits[:-max_waits]:
                        counter += 1
                        nop = mybir.InstNoOp(
                            name=f"I-waitsplit-{counter}", ins=[], outs=[]
                        )
                        nop.engine = inst.engine
                        nop.sync_info = bass_rust.SyncInfo(on_wait=[w], on_update=[])
                        new_list.append(nop)
                        nc.register_instruction(nop, overwrite=True)
                    inst.sync_info = bass_rust.SyncInfo(
                        on_wait=keep, on_update=list(si.on_update)
                    )
                    changed = True
                new_list.append(inst)
            if changed:
                il.clear()
                il.extend(new_list)
    return counter


def _mm(nc, out, lhsT, rhs, start, stop):
    nc.tensor.matmul(out, lhsT, rhs, start=start, stop=stop)


def build_nc():
    nc = bass.Bass()
    x_ext = nc.declare_dram_parameter("x", [C, L], MM_DT, isOutput=False)
    wq_ext = nc.declare_dram_parameter("wq", [C, HPC * D], MM_DT, isOutput=False)
    wk_ext = nc.declare_dram_parameter("wk", [C, HPC * D], MM_DT, isOutput=False)
    wv_ext = nc.declare_dram_parameter("wv", [C, HPC * D], MM_DT, isOutput=False)
    wo_ext = nc.declare_dram_parameter("wo", [HPC * D, C], MM_DT, isOutput=False)
    out_ext = nc.declare_dram_parameter("out", [C, L], MM_DT, isOutput=True)
    scratch = nc.dram_tensor("scratch", [HPC, L, D], MM_DT)

    NJ = L // 128  # 16 j tiles per head
    NIB = L // 512  # 4 i blocks per head
    NCC = C // 128  # 4 contraction chunks

    with tile.TileContext(nc) as tc:
        with (
            tc.tile_pool(name="const", bufs=1) as cpool,
            tc.tile_pool(name="exp", bufs=3) as epool,
            tc.tile_pool(name="o2", bufs=4) as o2pool,
            tc.tile_pool(name="rz", bufs=8) as rzpool,
            tc.tile_pool(name="fout", bufs=4) as fpool,
            tc.tile_pool(name="ps", bufs=2, space="PSUM") as ppool,
        ):
            # ---- persistent SBUF tensors ----
            # MDT tensors feed TensorE matmuls; float32r must be rounded
            # at the producing instruction (DMA/copy/activation output).
            MDT = MM_DT
            x_sbs = [cpool.tile([128, L], MDT, tag=f"x{ci}", name=f"x_sb{ci}") for ci in range(NCC)]
            wq_sb = cpool.tile([128, NCC, HPC * D], MDT, tag="wq")
            wk_sb = cpool.tile([128, NCC, HPC * D], MDT, tag="wk")
            wv_sb = cpool.tile([128, NCC, HPC * D], MDT, tag="wv")
            wo_sb = cpool.tile([128, 2, C], MDT, tag="wo")
            q_sbs = [cpool.tile([128, L], MDT, tag=f"q{g}", name=f"q_sb{g}") for g in range(2)]
            k_sbs = [cpool.tile([128, L], MDT, tag=f"k{g}", name=f"k_sb{g}") for g in range(2)]
            # vT1 split into 4 j-quarters so PV can start before all 16 done
            vT1s = [
                cpool.tile([128, 4, HPC, D + 1], MDT, tag=f"vT1{qt}", name=f"vT1_sb{qt}")
                for qt in range(4)
            ]
            out2_sb = cpool.tile([128, HPC, NJ, D], MDT, tag="out2")
            outrs_sbs = [cpool.tile([128, L], MDT, tag=f"outrs{g}", name=f"outrs_sb{g}") for g in range(2)]
            ident = cpool.tile([128, 128], MDT, tag="ident")

            masks.make_identity(nc, ident[:, :])
            ones_f32 = cpool.tile([128, 4 * HPC], FP, tag="ones")
            nc.vector.memset(ones_f32, 1.0)
            for qt in range(4):
                nc.vector.tensor_copy(
                    out=vT1s[qt][:, :, :, D : D + 1],
                    in_=ones_f32.rearrange("p (a b) -> p a b", b=HPC).unsqueeze(-1),
                )

            # ---- phase A: DMA inputs ----
            nc.sync.dma_start(
                out=wq_sb, in_=wq_ext.rearrange("(ci p) n -> p ci n", p=128)
            )
            nc.sync.dma_start(
                out=wk_sb, in_=wk_ext.rearrange("(ci p) n -> p ci n", p=128)
            )
            nc.sync.dma_start(
                out=wv_sb, in_=wv_ext.rearrange("(ci p) n -> p ci n", p=128)
            )
            nc.sync.dma_start(
                out=wo_sb, in_=wo_ext.rearrange("(rc p) o -> p rc o", p=128)
            )
            for ci in range(NCC):
                nc.sync.dma_start(
                    out=x_sbs[ci], in_=x_ext[ci * 128 : (ci + 1) * 128, :]
                )

            # ---- projection emitters (used pre-B and as in-block tasks) ----
            def qk_group(w_sb, g, dst, lb, eng="vector", ptag="s"):
                def t():
                    ps = ppool.tile([128, 512], FP, tag=ptag)
                    for ci in range(NCC):
                        _mm(
                            nc,
                            ps,
                            w_sb[:, ci, g * 128 : (g + 1) * 128],
                            x_sbs[ci][:, lb * 512 : (lb + 1) * 512],
                            start=(ci == 0),
                            stop=(ci == NCC - 1),
                        )
                    dsl = dst[:, lb * 512 : (lb + 1) * 512]
                    if eng == "scalar":
                        nc.scalar.copy(out=dsl, in_=ps)
                    else:
                        nc.vector.tensor_copy(out=dsl, in_=ps)
                return t

            def v_quarter(qt, ptag="s"):
                def t():
                    for j4 in range(4):
                        jt = qt * 4 + j4
                        ps = ppool.tile([128, HPC * D], FP, tag=ptag)
                        for ci in range(NCC):
                            _mm(
                                nc,
                                ps,
                                x_sbs[ci][:, jt * 128 : (jt + 1) * 128],
                                wv_sb[:, ci, :],
                                start=(ci == 0),
                                stop=(ci == NCC - 1),
                            )
                        nc.vector.tensor_copy(
                            out=vT1s[qt][:, j4, :, 0:D],
                            in_=ps.rearrange("p (h d) -> p h d", h=HPC),
                        )
                return t

            # minimal pre-B work: enough q/k/vT for block 0's first half
            qk_group(wq_sb, 0, q_sbs[0], 0, "scalar")()
            qk_group(wk_sb, 0, k_sbs[0], 0, "vector")()
            qk_group(wq_sb, 0, q_sbs[0], 1, "scalar")()
            qk_group(wk_sb, 0, k_sbs[0], 1, "vector")()
            v_quarter(0)()
            v_quarter(1)()

            # per-block deferred tasks: {jt: emitter}.  Needed-by deadlines:
            # k0lb2/vTq2 by jt8 of block 0, k0lb3/vTq3 by jt12 of block 0,
            # q0lbN by block N start, q1/k1 by block 4 start.
            block_tasks = {
                0: {3: qk_group(wk_sb, 0, k_sbs[0], 2, ptag="t"),
                    6: v_quarter(2, ptag="t"),
                    9: qk_group(wk_sb, 0, k_sbs[0], 3, ptag="t"),
                    12: v_quarter(3, ptag="t")},
                1: {3: qk_group(wq_sb, 0, q_sbs[0], 2, ptag="t"),
                    7: qk_group(wq_sb, 1, q_sbs[1], 0, ptag="t"),
                    10: qk_group(wq_sb, 1, q_sbs[1], 1, ptag="t"),
                    13: qk_group(wq_sb, 0, q_sbs[0], 3, ptag="t")},
                2: {3: qk_group(wq_sb, 1, q_sbs[1], 2, ptag="t"),
                    7: qk_group(wq_sb, 1, q_sbs[1], 3, ptag="t"),
                    10: qk_group(wk_sb, 1, k_sbs[1], 0, ptag="t"),
                    13: qk_group(wk_sb, 1, k_sbs[1], 1, ptag="t")},
                3: {3: qk_group(wk_sb, 1, k_sbs[1], 2, ptag="t"),
                    7: qk_group(wk_sb, 1, k_sbs[1], 3, ptag="t")},
            }

            # ---- phase B/C: attention per (head-pair, i-block) ----
            # Two heads of a pair sit on partition halves 0:64 / 64:128, so
            # their K=64 S^T matmuls row-tile into disjoint PE quadrants and
            # run concurrently.  Each block's transpose/normalize tail is
            # split into 2-transpose chunks spread over the NEXT block so the
            # PE never runs >2 consecutive non-S^T ops while ACT needs food.
            scratch_rs = scratch.ap().flatten().rearrange("(q e) -> q e", e=L)
            pending = []

            def make_flush_unit(h, ib2, cc, o2T):
                def u():
                    tg = ib2 * 4 + cc
                    ps_t = ppool.tile([128, 512], MDT, tag="t")
                    nc.tensor.transpose(
                        ps_t[:, 0:128],
                        o2T[:, cc * 128 : (cc + 1) * 128],
                        ident[:, :],
                    )
                    rz = rzpool.tile([128, 1], FP, tag="rz")
                    nc.vector.reciprocal(out=rz, in_=ps_t[:, D : D + 1])
                    nc.vector.tensor_scalar_mul(
                        out=out2_sb[:, h, tg, :],
                        in0=ps_t[:, 0:D],
                        scalar1=rz,
                    )
                    # contiguous 16KB write: scratch rows [tg*128, tg*128+128)
                    nc.sync.dma_start(
                        out=scratch[h, tg * 128 : (tg + 1) * 128, :],
                        in_=out2_sb[:, h, tg, :],
                    )
                    if cc == 3 and h == 1 and ib2 == NIB - 1:
                        nc.sync.dma_start(
                            out=outrs_sbs[0], in_=scratch_rs[0:128, :]
                        )
                return u

            FLUSH_JTS = (2, 5, 8, 11)
            for g in range(2):
                for ib in range(NIB):
                    bi = g * NIB + ib
                    tasks = block_tasks.get(bi, {})
                    i0 = ib * 512
                    ps_oA = ppool.tile([128, 512], FP, tag="o")
                    ps_oB = ppool.tile([128, 512], FP, tag="o")
                    for jt in range(NJ):
                        if jt in FLUSH_JTS:
                            for _ in range(2):
                                if pending:
                                    pending.pop(0)()
                        if jt in tasks:
                            tasks[jt]()
                        ps_s = ppool.tile([128, 1024], FP, tag="s")
                        for hp in range(2):
                            p0 = hp * 64
                            _mm(
                                nc,
                                ps_s[:, hp * 512 : (hp + 1) * 512],
                                k_sbs[g][p0 : p0 + 64, jt * 128 : (jt + 1) * 128],
                                q_sbs[g][p0 : p0 + 64, i0 : i0 + 512],
                                start=True,
                                stop=True,
                            )
                        ex = epool.tile([128, 1024], MDT, tag="exp")
                        nc.scalar.activation(
                            out=ex, in_=ps_s, func=mybir.ActivationFunctionType.Exp
                        )
                        for hp, ps_o in ((0, ps_oA), (1, ps_oB)):
                            _mm(
                                nc,
                                ps_o[0 : D + 1, :],
                                vT1s[jt // 4][:, jt % 4, 2 * g + hp, :],
                                ex[:, hp * 512 : (hp + 1) * 512],
                                start=(jt == 0),
                                stop=(jt == NJ - 1),
                            )
                    # copy accumulators out promptly to free the PSUM banks
                    for hp, ps_o in ((0, ps_oA), (1, ps_oB)):
                        h = 2 * g + hp
                        o2T = o2pool.tile([128, 512], MDT, tag="o2T")
                        nc.vector.tensor_copy(
                            out=o2T[0 : D + 1, :], in_=ps_o[0 : D + 1, :]
                        )
                        for cc in range(4):
                            pending.append(make_flush_unit(h, ib, cc, o2T))
            while pending:
                pending.pop(0)()
            nc.sync.dma_start(out=outrs_sbs[1], in_=scratch_rs[128:256, :])

            # ---- phase D: output projection on the reshaped rows ----
            for og in range(4):
                for lb in range(NIB):
                    it = og * NIB + lb
                    ps_f = ppool.tile(
                        [128, 512], FP, tag="o" if it % 2 == 0 else "t"
                    )
                    for rc in range(2):
                        _mm(
                            nc,
                            ps_f,
                            wo_sb[:, rc, og * 128 : (og + 1) * 128],
                            outrs_sbs[rc][:, lb * 512 : (lb + 1) * 512],
                            start=(rc == 0),
                            stop=(rc == 1),
                        )
                    fo = fpool.tile([128, 512], MDT, tag="fout")
                    if it % 2 == 0:
                        nc.vector.tensor_copy(out=fo, in_=ps_f)
                    else:
                        nc.scalar.copy(out=fo, in_=ps_f)
                    nc.sync.dma_start(
                        out=out_ext[
                            og * 128 : (og + 1) * 128, lb * 512 : (lb + 1) * 512
                        ],
                        in_=fo,
                    )

    _split_waits(nc)
    return nc


def _get_nc():
    key = str(MM_DT)
    if key not in _NC_CACHE:
        _NC_CACHE[key] = build_nc()
    return _NC_CACHE[key]


def kernel(x, w_qkv, w_out, b_out):
    global LAST_RESULT
    x = np.asarray(x, dtype=np.float32)
    w_qkv = np.asarray(w_qkv, dtype=np.float32)
    w_out = np.asarray(w_out, dtype=np.float32)
    b_out = np.asarray(b_out, dtype=np.float32)

    scale = D**-0.5
    in_maps = []
    for m in range(NCORES):
        b = m // 2
        hs = [4 * (m % 2) + i for i in range(HPC)]
        q_rows = np.concatenate([np.arange(h * D, (h + 1) * D) for h in hs])
        wq = np.ascontiguousarray((w_qkv[q_rows, :] * scale).T)
        wk = np.ascontiguousarray(w_qkv[C + q_rows, :].T)
        wv = np.ascontiguousarray(w_qkv[2 * C + q_rows, :].T)
        wo = np.ascontiguousarray(w_out[:, q_rows].T)
        bf16 = ml_dtypes.bfloat16
        in_maps.append(
            {
                "x": np.ascontiguousarray(x[b]).astype(bf16),
                "wq": wq.astype(bf16),
                "wk": wk.astype(bf16),
                "wv": wv.astype(bf16),
                "wo": wo.astype(bf16),
            }
        )

    nc = _get_nc()
    res = run_bass_kernel_spmd(
        nc, in_maps, core_ids=list(range(NCORES)), trace=TRACE_MODE
    )
    LAST_RESULT = res

    out = np.empty((B, C, L), dtype=np.float32)
    for b in range(B):
        out[b] = res.results[2 * b]["out"].astype(np.float32) + res.results[
            2 * b + 1
        ]["out"].astype(np.float32)
        out[b] += b_out[:, None]
    return out


# revision 23
# speedup vs baseline: 1.0083x; 1.0083x over previous
"""Distributed Trainium2 kernel for nn_Attention_72722386256499.

Full inputs in, full output out.  Internally shards the 32 (B,H)
attention problems over 8 NeuronCores: core m handles batch m//2,
heads [4*(m%2), 4*(m%2)+4).  The small 1x1-conv weights are sliced and
replicated host-side; the output projection is computed as per-core
partial products summed on the host (data-parallel reduce in unshard).
"""

import sys

sys.path.insert(0, "/opt/trn_rl_repo")

import ml_dtypes
import numpy as np

import bass_rust
import concourse.bass as bass
import concourse.mybir as mybir
import concourse.tile as tile
from concourse import masks
from concourse.bass_utils import run_bass_kernel_spmd

B, C, L = 4, 512, 2048
H, D = 8, 64
HPC = 4  # heads per core
NCORES = 8
FP = mybir.dt.float32

# Matmul compute dtype: float32 (4 cyc/row) or float32r (1 cyc/row @ N>=256).
MM_DT = mybir.dt.bfloat16

TRACE_MODE = False
LAST_RESULT = None
_NC_CACHE = {}


def _split_waits(nc, max_waits=1):
    """walrus here rejects >1 sync wait per instruction; hoist extras onto
    single-wait NoOps just before the instruction on the same engine."""
    counter = 0
    for f in nc.m.functions:
        for bb in f.blocks:
            il = bb.instructions
            new_list = []
            changed = False
            for inst in il:
                si = inst.sync_info
                if si is None:
                    new_list.append(inst)
                    continue
                waits = list(si.on_wait)
                if len(waits) > max_waits:
                    keep = waits[-max_waits:]
                    for w in waits[:-max_waits]:
                        counter += 1
                        nop = mybir.InstNoOp(
                            name=f"I-waitsplit-{counter}", ins=[], outs=[]
                        )
                        nop.engine = inst.engine
                        nop.sync_info = bass_rust.SyncInfo(on_wait=[w], on_update=[])
                        new_list.append(nop)
                        nc.register_instruction(nop, overwrite=True)
                    inst.sync_info = bass_rust.SyncInfo(
                        on_wait=keep, on_update=list(si.on_update)
                    )
                    changed = True
                new_list.append(inst)
            if changed:
                il.clear()
                il.extend(new_list)
    return counter


def _mm(nc, out, lhsT, rhs, start, stop):
    nc.tensor.matmul(out, lhsT, rhs, start=start, stop=stop)


def build_nc():
    nc = bass.Bass()
    x_ext = nc.declare_dram_parameter("x", [C, L], MM_DT, isOutput=False)
    wq_ext = nc.declare_dram_parameter("wq", [C, HPC * D], MM_DT, isOutput=False)
    wk_ext = nc.declare_dram_parameter("wk", [C, HPC * D], MM_DT, isOutput=False)
    wv_ext = nc.declare_dram_parameter("wv", [C, HPC * D], MM_DT, isOutput=False)
    wo_ext = nc.declare_dram_parameter("wo", [HPC * D, C], MM_DT, isOutput=False)
    out_ext = nc.declare_dram_parameter("out", [C, L], MM_DT, isOutput=True)
    scratch = nc.dram_tensor("scratch", [HPC, L, D], MM_DT)

    NJ = L // 128  # 16 j tiles per head
    NIB = L // 512  # 4 i blocks per head
    NCC = C // 128  # 4 contraction chunks

    with tile.TileContext(nc) as tc:
        with (
            tc.tile_pool(name="const", bufs=1) as cpool,
            tc.tile_pool(name="exp", bufs=3) as epool,
            tc.tile_pool(name="o2", bufs=4) as o2pool,
            tc.tile_pool(name="rz", bufs=8) as rzpool,
            tc.tile_pool(name="fout", bufs=4) as fpool,
            tc.tile_pool(name="ps", bufs=2, space="PSUM") as ppool,
        ):
            # ---- persistent SBUF tensors ----
            # MDT tensors feed TensorE matmuls; float32r must be rounded
            # at the producing instruction (DMA/copy/activation output).
            MDT = MM_DT
            x_sbs = [cpool.tile([128, L], MDT, tag=f"x{ci}", name=f"x_sb{ci}") for ci in range(NCC)]
            wq_sb = cpool.tile([128, NCC, HPC * D], MDT, tag="wq")
            wk_sb = cpool.tile([128, NCC, HPC * D], MDT, tag="wk")
            wv_sb = cpool.tile([128, NCC, HPC * D], MDT, tag="wv")
            wo_sb = cpool.tile([128, 2, C], MDT, tag="wo")
            q_sbs = [cpool.tile([128, L], MDT, tag=f"q{g}", name=f"q_sb{g}") for g in range(2)]
            k_sbs = [cpool.tile([128, L], MDT, tag=f"k{g}", name=f"k_sb{g}") for g in range(2)]
            # vT1 split into 4 j-quarters so PV can start before all 16 done
            vT1s = [
                cpool.tile([128, 4, HPC, D + 1], MDT, tag=f"vT1{qt}", name=f"vT1_sb{qt}")
                for qt in range(4)
            ]
            out2_sb = cpool.tile([128, HPC, NJ, D], MDT, tag="out2")
            outrs_sbs = [cpool.tile([128, L], MDT, tag=f"outrs{g}", name=f"outrs_sb{g}") for g in range(2)]
            ident = cpool.tile([128, 128], MDT, tag="ident")

            masks.make_identity(nc, ident[:, :])
            ones_f32 = cpool.tile([128, 4 * HPC], FP, tag="ones")
            nc.vector.memset(ones_f32, 1.0)
            for qt in range(4):
                nc.vector.tensor_copy(
                    out=vT1s[qt][:, :, :, D : D + 1],
                    in_=ones_f32.rearrange("p (a b) -> p a b", b=HPC).unsqueeze(-1),
                )

            # ---- phase A: DMA inputs ----
            nc.sync.dma_start(
                out=wq_sb, in_=wq_ext.rearrange("(ci p) n -> p ci n", p=128)
            )
            nc.sync.dma_start(
                out=wk_sb, in_=wk_ext.rearrange("(ci p) n -> p ci n", p=128)
            )
            nc.sync.dma_start(
                out=wv_sb, in_=wv_ext.rearrange("(ci p) n -> p ci n", p=128)
            )
            nc.sync.dma_start(
                out=wo_sb, in_=wo_ext.rearrange("(rc p) o -> p rc o", p=128)
            )
            for ci in range(NCC):
                nc.sync.dma_start(
                    out=x_sbs[ci], in_=x_ext[ci * 128 : (ci + 1) * 128, :]
                )

            # ---- projection emitters (used pre-B and as in-block tasks) ----
            def qk_group(w_sb, g, dst, lb, eng="vector", ptag="s"):
                def t():
                    ps = ppool.tile([128, 512], FP, tag=ptag)
                    for ci in range(NCC):
                        _mm(
                            nc,
                            ps,
                            w_sb[:, ci, g * 128 : (g + 1) * 128],
                            x_sbs[ci][:, lb * 512 : (lb + 1) * 512],
                            start=(ci == 0),
                            stop=(ci == NCC - 1),
                        )
                    dsl = dst[:, lb * 512 : (lb + 1) * 512]
                    if eng == "scalar":
                        nc.scalar.copy(out=dsl, in_=ps)
                    else:
                        nc.vector.tensor_copy(out=dsl, in_=ps)
                return t

            def v_quarter(qt, ptag="s"):
                def t():
                    for j4 in range(4):
                        jt = qt * 4 + j4
                        ps = ppool.tile([128, HPC * D], FP, tag=ptag)
                        for ci in range(NCC):
                            _mm(
                                nc,
                                ps,
                                x_sbs[ci][:, jt * 128 : (jt + 1) * 128],
                                wv_sb[:, ci, :],
                                start=(ci == 0),
                                stop=(ci == NCC - 1),
                            )
                        nc.vector.tensor_copy(
                            out=vT1s[qt][:, j4, :, 0:D],
                            in_=ps.rearrange("p (h d) -> p h d", h=HPC),
                        )
                return t

            # minimal pre-B work: enough q/k/vT for block 0's first half
            qk_group(wq_sb, 0, q_sbs[0], 0, "scalar")()
            qk_group(wk_sb, 0, k_sbs[0], 0, "vector")()
            qk_group(wq_sb, 0, q_sbs[0], 1, "scalar")()
            qk_group(wk_sb, 0, k_sbs[0], 1, "vector")()
            v_quarter(0)()
            v_quarter(1)()

            # per-block deferred tasks: {jt: emitter}.  Needed-by deadlines:
            # k0lb2/vTq2 by jt8 of block 0, k0lb3/vTq3 by jt12 of block 0,
            # q0lbN by block N start, q1/k1 by block 4 start.
            block_tasks = {
                0: {3: qk_group(wk_sb, 0, k_sbs[0], 2, ptag="t"),
                    6: v_quarter(2, ptag="t"),
                    9: qk_group(wk_sb, 0, k_sbs[0], 3, ptag="t"),
                    12: v_quarter(3, ptag="t")},
                1: {3: qk_group(wq_sb, 0, q_sbs[0], 2, ptag="t"),
                    7: qk_group(wq_sb, 1, q_sbs[1], 0, ptag="t"),
                    10: qk_group(wq_sb, 1, q_sbs[1], 1, ptag="t"),
                    13: qk_group(wq_sb, 0, q_sbs[0], 3, ptag="t")},
                2: {3: qk_group(wq_sb, 1, q_sbs[1], 2, ptag="t"),
                    7: qk_group(wq_sb, 1, q_sbs[1], 3, ptag="t"),
                    10: qk_group(wk_sb, 1, k_sbs[1], 0, ptag="t"),
                    13: qk_group(wk_sb, 1, k_sbs[1], 1, ptag="t")},
                3: {3: qk_group(wk_sb, 1, k_sbs[1], 2, ptag="t"),
                    7: qk_group(wk_sb, 1, k_sbs[1], 3, ptag="t")},
            }

            # ---- phase B/C: attention per (head-pair, i-block) ----
            # Two heads of a pair sit on partition halves 0:64 / 64:128, so
            # their K=64 S^T matmuls row-tile into disjoint PE quadrants and
            # run concurrently.  Each block's transpose/normalize tail is
            # split into 2-transpose chunks spread over the NEXT block so the
            # PE never runs >2 consecutive non-S^T ops while ACT needs food.
            scratch_rs = scratch.ap().flatten().rearrange("(q e) -> q e", e=L)
            pending = []

            def make_flush_unit(h, ib2, cc, o2T):
                def u():
                    tg = ib2 * 4 + cc
                    ps_t = ppool.tile([128, 512], MDT, tag="t")
                    nc.tensor.transpose(
                        ps_t[:, 0:128],
                        o2T[:, cc * 128 : (cc + 1) * 128],
                        ident[:, :],
                    )
                    rz = rzpool.tile([128, 1], FP, tag="rz")
                    nc.vector.reciprocal(out=rz, in_=ps_t[:, D : D + 1])
                    nc.vector.tensor_scalar_mul(
                        out=out2_sb[:, h, tg, :],
                        in0=ps_t[:, 0:D],
                        scalar1=rz,
                    )
                    if h >= 2 and ib2 == NIB - 1:
                        # final block: latency-critical -> contiguous 16KB
                        # write per unit, spread across DMA queues
                        nc.sync.dma_start(
                            out=scratch[h, tg * 128 : (tg + 1) * 128, :],
                            in_=out2_sb[:, h, tg, :],
                        )
                    elif cc == 3:
                        nc.sync.dma_start(
                            out=scratch[
                                h, ib2 * 512 : (ib2 + 1) * 512, :
                            ].rearrange("(c2 p) d -> p c2 d", p=128),
                            in_=out2_sb[:, h, ib2 * 4 : (ib2 + 1) * 4, :],
                        )
                    if cc == 3 and h == 1 and ib2 == NIB - 1:
                        nc.sync.dma_start(
                            out=outrs_sbs[0], in_=scratch_rs[0:128, :]
                        )
                return u

            FLUSH_JTS = (2, 5, 8, 11)
            for g in range(2):
                for ib in range(NIB):
                    bi = g * NIB + ib
                    tasks = block_tasks.get(bi, {})
                    i0 = ib * 512
                    ps_oA = ppool.tile([128, 512], FP, tag="o")
                    ps_oB = ppool.tile([128, 512], FP, tag="o")
                    for jt in range(NJ):
                        if jt in FLUSH_JTS:
                            for _ in range(2):
                                if pending:
                                    pending.pop(0)()
                        if jt in tasks:
                            tasks[jt]()
                        ps_s = ppool.tile([128, 1024], FP, tag="s")
                        for hp in range(2):
                            p0 = hp * 64
                            _mm(
                                nc,
                                ps_s[:, hp * 512 : (hp + 1) * 512],
                                k_sbs[g][p0 : p0 + 64, jt * 128 : (jt + 1) * 128],
                                q_sbs[g][p0 : p0 + 64, i0 : i0 + 512],
                                start=True,
                                stop=True,
                            )
                        ex = epool.tile([128, 1024], MDT, tag="exp")
                        nc.scalar.activation(
                            out=ex, in_=ps_s, func=mybir.ActivationFunctionType.Exp
                        )
                        for hp, ps_o in ((0, ps_oA), (1, ps_oB)):
                            _mm(
                                nc,
                                ps_o[0 : D + 1, :],
                                vT1s[jt // 4][:, jt % 4, 2 * g + hp, :],
                                ex[:, hp * 512 : (hp + 1) * 512],
                                start=(jt == 0),
                                stop=(jt == NJ - 1),
                            )
                    # copy accumulators out promptly to free the PSUM banks
                    for hp, ps_o in ((0, ps_oA), (1, ps_oB)):
                        h = 2 * g + hp
                        o2T = o2pool.tile([128, 512], MDT, tag="o2T")
                        nc.vector.tensor_copy(
                            out=o2T[0 : D + 1, :], in_=ps_o[0 : D + 1, :]
                        )
                        for cc in range(4):
                            pending.append(make_flush_unit(h, ib, cc, o2T))
            while pending:
                pending.pop(0)()
            nc.sync.dma_start(out=outrs_sbs[1], in_=scratch_rs[128:256, :])

            # ---- phase D: output projection on the reshaped rows ----
            for og in range(4):
                for lb in range(NIB):
                    it = og * NIB + lb
                    ps_f = ppool.tile(
                        [128, 512], FP, tag="o" if it % 2 == 0 else "t"
                    )
                    for rc in range(2):
                        _mm(
                            nc,
                            ps_f,
                            wo_sb[:, rc, og * 128 : (og + 1) * 128],
                            outrs_sbs[rc][:, lb * 512 : (lb + 1) * 512],
                            start=(rc == 0),
                            stop=(rc == 1),
                        )
                    fo = fpool.tile([128, 512], MDT, tag="fout")
                    if it % 2 == 0:
                        nc.vector.tensor_copy(out=fo, in_=ps_f)
                    else:
                        nc.scalar.copy(out=fo, in_=ps_f)
                    nc.sync.dma_start(
                        out=out_ext[
                            og * 128 : (og + 1) * 128, lb * 512 : (lb + 1) * 512
                        ],
                        in_=fo,
                    )

    _split_waits(nc)
    return nc


def _get_nc():
    key = str(MM_DT)
    if key not in _NC_CACHE:
        _NC_CACHE[key] = build_nc()
    return _NC_CACHE[key]


def kernel(x, w_qkv, w_out, b_out):
    global LAST_RESULT
    x = np.asarray(x, dtype=np.float32)
    w_qkv = np.asarray(w_qkv, dtype=np.float32)
    w_out = np.asarray(w_out, dtype=np.float32)
    b_out = np.asarray(b_out, dtype=np.float32)

    scale = D**-0.5
    in_maps = []
    for m in range(NCORES):
        b = m // 2
        hs = [4 * (m % 2) + i for i in range(HPC)]
        q_rows = np.concatenate([np.arange(h * D, (h + 1) * D) for h in hs])
        wq = np.ascontiguousarray((w_qkv[q_rows, :] * scale).T)
        wk = np.ascontiguousarray(w_qkv[C + q_rows, :].T)
        wv = np.ascontiguousarray(w_qkv[2 * C + q_rows, :].T)
        wo = np.ascontiguousarray(w_out[:, q_rows].T)
        bf16 = ml_dtypes.bfloat16
        in_maps.append(
            {
                "x": np.ascontiguousarray(x[b]).astype(bf16),
                "wq": wq.astype(bf16),
                "wk": wk.astype(bf16),
                "wv": wv.astype(bf16),
                "wo": wo.astype(bf16),
            }
        )

    nc = _get_nc()
    res = run_bass_kernel_spmd(
        nc, in_maps, core_ids=list(range(NCORES)), trace=TRACE_MODE
    )
    LAST_RESULT = res

    out = np.empty((B, C, L), dtype=np.float32)
    for b in range(B):
        out[b] = res.results[2 * b]["out"].astype(np.float32) + res.results[
            2 * b + 1
        ]["out"].astype(np.float32)
        out[b] += b_out[:, None]
    return out


# revision 24
# speedup vs baseline: 1.0146x; 1.0063x over previous
"""Distributed Trainium2 kernel for nn_Attention_72722386256499.

Full inputs in, full output out.  Internally shards the 32 (B,H)
attention problems over 8 NeuronCores: core m handles batch m//2,
heads [4*(m%2), 4*(m%2)+4).  The small 1x1-conv weights are sliced and
replicated host-side; the output projection is computed as per-core
partial products summed on the host (data-parallel reduce in unshard).
"""

import sys

sys.path.insert(0, "/opt/trn_rl_repo")

import ml_dtypes
import numpy as np

import bass_rust
import concourse.bass as bass
import concourse.mybir as mybir
import concourse.tile as tile
from concourse import masks
from concourse.bass_utils import run_bass_kernel_spmd

B, C, L = 4, 512, 2048
H, D = 8, 64
HPC = 4  # heads per core
NCORES = 8
FP = mybir.dt.float32

# Matmul compute dtype: float32 (4 cyc/row) or float32r (1 cyc/row @ N>=256).
MM_DT = mybir.dt.bfloat16

TRACE_MODE = False
LAST_RESULT = None
_NC_CACHE = {}


def _split_waits(nc, max_waits=1):
    """walrus here rejects >1 sync wait per instruction; hoist extras onto
    single-wait NoOps just before the instruction on the same engine."""
    counter = 0
    for f in nc.m.functions:
        for bb in f.blocks:
            il = bb.instructions
            new_list = []
            changed = False
            for inst in il:
                si = inst.sync_info
                if si is None:
                    new_list.append(inst)
                    continue
                waits = list(si.on_wait)
                if len(waits) > max_waits:
                    keep = waits[-max_waits:]
                    for w in waits[:-max_waits]:
                        counter += 1
                        nop = mybir.InstNoOp(
                            name=f"I-waitsplit-{counter}", ins=[], outs=[]
                        )
                        nop.engine = inst.engine
                        nop.sync_info = bass_rust.SyncInfo(on_wait=[w], on_update=[])
                        new_list.append(nop)
                        nc.register_instruction(nop, overwrite=True)
                    inst.sync_info = bass_rust.SyncInfo(
                        on_wait=keep, on_update=list(si.on_update)
                    )
                    changed = True
                new_list.append(inst)
            if changed:
                il.clear()
                il.extend(new_list)
    return counter


def _mm(nc, out, lhsT, rhs, start, stop):
    nc.tensor.matmul(out, lhsT, rhs, start=start, stop=stop)


def build_nc():
    nc = bass.Bass()
    x_ext = nc.declare_dram_parameter("x", [C, L], MM_DT, isOutput=False)
    wq_ext = nc.declare_dram_parameter("wq", [C, HPC * D], MM_DT, isOutput=False)
    wk_ext = nc.declare_dram_parameter("wk", [C, HPC * D], MM_DT, isOutput=False)
    wv_ext = nc.declare_dram_parameter("wv", [C, HPC * D], MM_DT, isOutput=False)
    wo_ext = nc.declare_dram_parameter("wo", [HPC * D, C], MM_DT, isOutput=False)
    out_ext = nc.declare_dram_parameter("out", [C, L], MM_DT, isOutput=True)
    scratch = nc.dram_tensor("scratch", [HPC, L, D], MM_DT)

    NJ = L // 128  # 16 j tiles per head
    NIB = L // 512  # 4 i blocks per head
    NCC = C // 128  # 4 contraction chunks

    with tile.TileContext(nc) as tc:
        with (
            tc.tile_pool(name="const", bufs=1) as cpool,
            tc.tile_pool(name="exp", bufs=3) as epool,
            tc.tile_pool(name="o2", bufs=4) as o2pool,
            tc.tile_pool(name="rz", bufs=8) as rzpool,
            tc.tile_pool(name="fout", bufs=6) as fpool,
            tc.tile_pool(name="ps", bufs=2, space="PSUM") as ppool,
        ):
            # ---- persistent SBUF tensors ----
            # MDT tensors feed TensorE matmuls; float32r must be rounded
            # at the producing instruction (DMA/copy/activation output).
            MDT = MM_DT
            x_sbs = [
                [
                    cpool.tile(
                        [128, 512], MDT, tag=f"x{ci}_{lb}", name=f"x_sb{ci}_{lb}"
                    )
                    for lb in range(NIB)
                ]
                for ci in range(NCC)
            ]
            wq_sb = cpool.tile([128, NCC, HPC * D], MDT, tag="wq")
            wk_sb = cpool.tile([128, NCC, HPC * D], MDT, tag="wk")
            wv_sb = cpool.tile([128, NCC, HPC * D], MDT, tag="wv")
            wo_sb = cpool.tile([128, 2, C], MDT, tag="wo")
            q_sbs = [cpool.tile([128, L], MDT, tag=f"q{g}", name=f"q_sb{g}") for g in range(2)]
            k_sbs = [cpool.tile([128, L], MDT, tag=f"k{g}", name=f"k_sb{g}") for g in range(2)]
            # vT1 split into 4 j-quarters so PV can start before all 16 done
            vT1s = [
                cpool.tile([128, 4, HPC, D + 1], MDT, tag=f"vT1{qt}", name=f"vT1_sb{qt}")
                for qt in range(4)
            ]
            out2_sb = cpool.tile([128, HPC, NJ, D], MDT, tag="out2")
            outrs_sbs = [cpool.tile([128, L], MDT, tag=f"outrs{g}", name=f"outrs_sb{g}") for g in range(2)]
            ident = cpool.tile([128, 128], MDT, tag="ident")

            # input DMAs first so transfers start during the preamble;
            # lb-major order so block 0's columns land first
            nc.sync.dma_start(
                out=wq_sb, in_=wq_ext.rearrange("(ci p) n -> p ci n", p=128)
            )
            nc.sync.dma_start(
                out=wk_sb, in_=wk_ext.rearrange("(ci p) n -> p ci n", p=128)
            )
            for lb in range(NIB):
                for ci in range(NCC):
                    nc.sync.dma_start(
                        out=x_sbs[ci][lb],
                        in_=x_ext[
                            ci * 128 : (ci + 1) * 128, lb * 512 : (lb + 1) * 512
                        ],
                    )
            nc.sync.dma_start(
                out=wv_sb, in_=wv_ext.rearrange("(ci p) n -> p ci n", p=128)
            )
            nc.sync.dma_start(
                out=wo_sb, in_=wo_ext.rearrange("(rc p) o -> p rc o", p=128)
            )

            masks.make_identity(nc, ident[:, :])
            ones_f32 = cpool.tile([128, 4 * HPC], FP, tag="ones")
            nc.vector.memset(ones_f32, 1.0)
            for qt in range(4):
                nc.vector.tensor_copy(
                    out=vT1s[qt][:, :, :, D : D + 1],
                    in_=ones_f32.rearrange("p (a b) -> p a b", b=HPC).unsqueeze(-1),
                )

            # ---- projection emitters (used pre-B and as in-block tasks) ----
            def qk_group(w_sb, g, dst, lb, eng="vector", ptag="s"):
                def t():
                    ps = ppool.tile([128, 512], FP, tag=ptag)
                    for ci in range(NCC):
                        _mm(
                            nc,
                            ps,
                            w_sb[:, ci, g * 128 : (g + 1) * 128],
                            x_sbs[ci][lb][:, :],
                            start=(ci == 0),
                            stop=(ci == NCC - 1),
                        )
                    dsl = dst[:, lb * 512 : (lb + 1) * 512]
                    if eng == "scalar":
                        nc.scalar.copy(out=dsl, in_=ps)
                    else:
                        nc.vector.tensor_copy(out=dsl, in_=ps)
                return t

            def v_quarter(qt, ptag="s"):
                def t():
                    for j4 in range(4):
                        jt = qt * 4 + j4
                        ps = ppool.tile([128, HPC * D], FP, tag=ptag)
                        for ci in range(NCC):
                            _mm(
                                nc,
                                ps,
                                x_sbs[ci][jt // 4][
                                    :, (jt % 4) * 128 : (jt % 4 + 1) * 128
                                ],
                                wv_sb[:, ci, :],
                                start=(ci == 0),
                                stop=(ci == NCC - 1),
                            )
                        nc.vector.tensor_copy(
                            out=vT1s[qt][:, j4, :, 0:D],
                            in_=ps.rearrange("p (h d) -> p h d", h=HPC),
                        )
                return t

            # minimal pre-B work: enough q/k/vT for block 0's first half
            qk_group(wq_sb, 0, q_sbs[0], 0, "scalar")()
            qk_group(wk_sb, 0, k_sbs[0], 0, "vector")()
            qk_group(wq_sb, 0, q_sbs[0], 1, "scalar")()
            qk_group(wk_sb, 0, k_sbs[0], 1, "vector")()
            v_quarter(0)()
            v_quarter(1)()

            # per-block deferred tasks: {jt: emitter}.  Needed-by deadlines:
            # k0lb2/vTq2 by jt8 of block 0, k0lb3/vTq3 by jt12 of block 0,
            # q0lbN by block N start, q1/k1 by block 4 start.
            block_tasks = {
                0: {3: qk_group(wk_sb, 0, k_sbs[0], 2, ptag="t"),
                    6: v_quarter(2, ptag="t"),
                    9: qk_group(wk_sb, 0, k_sbs[0], 3, ptag="t"),
                    12: v_quarter(3, ptag="t")},
                1: {3: qk_group(wq_sb, 0, q_sbs[0], 2, ptag="t"),
                    7: qk_group(wq_sb, 1, q_sbs[1], 0, ptag="t"),
                    10: qk_group(wq_sb, 1, q_sbs[1], 1, ptag="t"),
                    13: qk_group(wq_sb, 0, q_sbs[0], 3, ptag="t")},
                2: {3: qk_group(wq_sb, 1, q_sbs[1], 2, ptag="t"),
                    7: qk_group(wq_sb, 1, q_sbs[1], 3, ptag="t"),
                    10: qk_group(wk_sb, 1, k_sbs[1], 0, ptag="t"),
                    13: qk_group(wk_sb, 1, k_sbs[1], 1, ptag="t")},
                3: {3: qk_group(wk_sb, 1, k_sbs[1], 2, ptag="t"),
                    7: qk_group(wk_sb, 1, k_sbs[1], 3, ptag="t")},
            }

            # ---- phase B/C: attention per (head-pair, i-block) ----
            # Two heads of a pair sit on partition halves 0:64 / 64:128, so
            # their K=64 S^T matmuls row-tile into disjoint PE quadrants and
            # run concurrently.  Each block's transpose/normalize tail is
            # split into 2-transpose chunks spread over the NEXT block so the
            # PE never runs >2 consecutive non-S^T ops while ACT needs food.
            scratch_rs = scratch.ap().flatten().rearrange("(q e) -> q e", e=L)
            pending = []

            def make_flush_unit(h, ib2, cc, o2T):
                def u():
                    tg = ib2 * 4 + cc
                    ps_t = ppool.tile([128, 512], MDT, tag="t")
                    nc.tensor.transpose(
                        ps_t[:, 0:128],
                        o2T[:, cc * 128 : (cc + 1) * 128],
                        ident[:, :],
                    )
                    rz = rzpool.tile([128, 1], FP, tag="rz")
                    nc.vector.reciprocal(out=rz, in_=ps_t[:, D : D + 1])
                    nc.vector.tensor_scalar_mul(
                        out=out2_sb[:, h, tg, :],
                        in0=ps_t[:, 0:D],
                        scalar1=rz,
                    )
                    if h >= 2 and ib2 == NIB - 1:
                        # final block: latency-critical -> contiguous 16KB
                        # write per unit, spread across DMA queues
                        nc.sync.dma_start(
                            out=scratch[h, tg * 128 : (tg + 1) * 128, :],
                            in_=out2_sb[:, h, tg, :],
                        )
                    elif cc == 3:
                        nc.sync.dma_start(
                            out=scratch[
                                h, ib2 * 512 : (ib2 + 1) * 512, :
                            ].rearrange("(c2 p) d -> p c2 d", p=128),
                            in_=out2_sb[:, h, ib2 * 4 : (ib2 + 1) * 4, :],
                        )
                    if cc == 3 and h == 1 and ib2 == NIB - 1:
                        nc.sync.dma_start(
                            out=outrs_sbs[0], in_=scratch_rs[0:128, :]
                        )
                return u

            FLUSH_JTS = (2, 5, 8, 11)
            for g in range(2):
                for ib in range(NIB):
                    bi = g * NIB + ib
                    tasks = block_tasks.get(bi, {})
                    i0 = ib * 512
                    ps_oA = ppool.tile([128, 512], FP, tag="o")
                    ps_oB = ppool.tile([128, 512], FP, tag="o")
                    for jt in range(NJ):
                        if jt in FLUSH_JTS:
                            for _ in range(2):
                                if pending:
                                    pending.pop(0)()
                        if jt in tasks:
                            tasks[jt]()
                        ps_s = ppool.tile([128, 1024], FP, tag="s")
                        for hp in range(2):
                            p0 = hp * 64
                            _mm(
                                nc,
                                ps_s[:, hp * 512 : (hp + 1) * 512],
                                k_sbs[g][p0 : p0 + 64, jt * 128 : (jt + 1) * 128],
                                q_sbs[g][p0 : p0 + 64, i0 : i0 + 512],
                                start=True,
                                stop=True,
                            )
                        ex = epool.tile([128, 1024], MDT, tag="exp")
                        nc.scalar.activation(
                            out=ex, in_=ps_s, func=mybir.ActivationFunctionType.Exp
                        )
                        for hp, ps_o in ((0, ps_oA), (1, ps_oB)):
                            _mm(
                                nc,
                                ps_o[0 : D + 1, :],
                                vT1s[jt // 4][:, jt % 4, 2 * g + hp, :],
                                ex[:, hp * 512 : (hp + 1) * 512],
                                start=(jt == 0),
                                stop=(jt == NJ - 1),
                            )
                    # copy accumulators out promptly to free the PSUM banks
                    for hp, ps_o in ((0, ps_oA), (1, ps_oB)):
                        h = 2 * g + hp
                        o2T = o2pool.tile([128, 512], MDT, tag="o2T")
                        nc.vector.tensor_copy(
                            out=o2T[0 : D + 1, :], in_=ps_o[0 : D + 1, :]
                        )
                        for cc in range(4):
                            pending.append(make_flush_unit(h, ib, cc, o2T))
            while pending:
                pending.pop(0)()
            nc.sync.dma_start(out=outrs_sbs[1], in_=scratch_rs[128:256, :])

            # ---- phase D: output projection on the reshaped rows ----
            for og in range(4):
                for lb in range(NIB):
                    it = og * NIB + lb
                    ps_f = ppool.tile(
                        [128, 512], FP, tag="o" if it % 2 == 0 else "t"
                    )
                    for rc in range(2):
                        _mm(
                            nc,
                            ps_f,
                            wo_sb[:, rc, og * 128 : (og + 1) * 128],
                            outrs_sbs[rc][:, lb * 512 : (lb + 1) * 512],
                            start=(rc == 0),
                            stop=(rc == 1),
                        )
                    fo = fpool.tile([128, 512], MDT, tag="fout")
                    if it % 2 == 0:
                        nc.vector.tensor_copy(out=fo, in_=ps_f)
                    else:
                        nc.scalar.copy(out=fo, in_=ps_f)
                    nc.sync.dma_start(
                        out=out_ext[
                            og * 128 : (og + 1) * 128, lb * 512 : (lb + 1) * 512
                        ],
                        in_=fo,
                    )

    _split_waits(nc)
    return nc


def _get_nc():
    key = str(MM_DT)
    if key not in _NC_CACHE:
        _NC_CACHE[key] = build_nc()
    return _NC_CACHE[key]


def kernel(x, w_qkv, w_out, b_out):
    global LAST_RESULT
    x = np.asarray(x, dtype=np.float32)
    w_qkv = np.asarray(w_qkv, dtype=np.float32)
    w_out = np.asarray(w_out, dtype=np.float32)
    b_out = np.asarray(b_out, dtype=np.float32)

    scale = D**-0.5
    in_maps = []
    for m in range(NCORES):
        b = m // 2
        hs = [4 * (m % 2) + i for i in range(HPC)]
        q_rows = np.concatenate([np.arange(h * D, (h + 1) * D) for h in hs])
        wq = np.ascontiguousarray((w_qkv[q_rows, :] * scale).T)
        wk = np.ascontiguousarray(w_qkv[C + q_rows, :].T)
        wv = np.ascontiguousarray(w_qkv[2 * C + q_rows, :].T)
        wo = np.ascontiguousarray(w_out[:, q_rows].T)
        bf16 = ml_dtypes.bfloat16
        in_maps.append(
            {
                "x": np.ascontiguousarray(x[b]).astype(bf16),
                "wq": wq.astype(bf16),
                "wk": wk.astype(bf16),
                "wv": wv.astype(bf16),
                "wo": wo.astype(bf16),
            }
        )

    nc = _get_nc()
    res = run_bass_kernel_spmd(
        nc, in_maps, core_ids=list(range(NCORES)), trace=TRACE_MODE
    )
    LAST_RESULT = res

    out = np.empty((B, C, L), dtype=np.float32)
    for b in range(B):
        out[b] = res.results[2 * b]["out"].astype(np.float32) + res.results[
            2 * b + 1
        ]["out"].astype(np.float32)
        out[b] += b_out[:, None]
    return out
